# revision 1
# baseline (speedup 1.0000x reference)
"""Trainium2 Bass kernel for the KvvGaussianLayer problem.

Computes, from tensor (B=8, T=2048, F=67) with F = [mean, z(64), v, noise]:
    mean    = tensor[:, :, 0]                                  (B, T)
    f_cov_j = exp(-0.5*||z_i/8 - z_j/8||^2) * v_i * v_j + J*I  (B, T, T)
    y_cov_j = f_cov_j + softplus(noise) * I                    (B, T, T)

Sharding: data-parallel over batch B across the 8 NeuronCores (batch b on
core b); each core builds its own (T, T) covariances independently.

Device strategy (per core):
  - Host augments z with 2 extra contraction rows so a single K=66 matmul
    produces quad[i,j] = z_i.z_j - 0.5||z_i||^2 - 0.5||z_j||^2 directly.
  - PE: 64 matmuls (128x512 out tiles, K=66, fp32) -> PSUM
  - ACT: exp(PSUM) -> SBUF
  - DVE: one fused scalar_tensor_tensor: (kern * v_i) * v_j, plus diagonal
    jitter/noise adds using an eye tile.
  - DMA out f and y as 1MB row-block transfers (memory-bound problem: the
    32 MB of per-core output writes are the roofline).
"""

import numpy as np

B = 8
T = 2048
D = 64
K = D + 2          # augmented contraction dim
JITTER = 1e-06
NB = T // 128      # 16 row blocks per core
NC4 = T // 512     # 4 column chunks per row block

_COMPILED = None


def _build_module():
    """Build and compile the per-core Bass/Tile module (SPMD: same program
    on all 8 cores, different data)."""
    import concourse.bacc as bacc
    import concourse.tile as tile
    from concourse import mybir

    f32 = mybir.dt.float32
    Alu = mybir.AluOpType
    Act = mybir.ActivationFunctionType

    nc = bacc.Bacc("TRN2", target_bir_lowering=False, debug=False)

    zlt = nc.dram_tensor("zlt", [K, T], f32, kind="ExternalInput").ap()
    zrt = nc.dram_tensor("zrt", [K, T], f32, kind="ExternalInput").ap()
    vrow = nc.dram_tensor("vrow", [1, T], f32, kind="ExternalInput").ap()
    vcol = nc.dram_tensor("vcol", [128, NB], f32, kind="ExternalInput").ap()
    ncol = nc.dram_tensor("ncol", [128, NB], f32, kind="ExternalInput").ap()
    ey = nc.dram_tensor("ey", [128, T], f32, kind="ExternalInput").ap()
    fj = nc.dram_tensor("fj", [T, T], f32, kind="ExternalOutput").ap()
    yj = nc.dram_tensor("yj", [T, T], f32, kind="ExternalOutput").ap()

    with tile.TileContext(nc) as tc:
        with (
            tc.tile_pool(name="singles", bufs=1) as singles,
            tc.tile_pool(name="tpool", bufs=6) as tpool,
            tc.tile_pool(name="ydpool", bufs=6) as ydpool,
            tc.tile_pool(name="psum", bufs=8, space="PSUM") as psum,
        ):
            s_zlt = singles.tile([K, T], f32)
            s_zrt = singles.tile([K, T], f32)
            s_vrow = singles.tile([128, T], f32)
            s_vcol = singles.tile([128, NB], f32)
            s_ncol = singles.tile([128, NB], f32)
            s_ey = singles.tile([128, T], f32)

            nc.sync.dma_start(out=s_zlt[:], in_=zlt[:])
            nc.sync.dma_start(out=s_zrt[:], in_=zrt[:])
            nc.sync.dma_start(out=s_vrow[:], in_=vrow.to_broadcast([128, T]))
            nc.sync.dma_start(out=s_vcol[:], in_=vcol[:])
            nc.sync.dma_start(out=s_ncol[:], in_=ncol[:])
            nc.sync.dma_start(out=s_ey[:], in_=ey[:])

            for i in range(NB):
                t = tpool.tile([128, T], f32)
                j0 = i // 4          # column chunk containing the diagonal
                m = i % 4            # eye offset variant (diag at col 128*m)
                c = 128 * i          # diagonal column within the row block

                for j in range(NC4):
                    ps = psum.tile([128, 512], f32)
                    nc.tensor.matmul(
                        ps[:],
                        lhsT=s_zlt[:, 128 * i : 128 * (i + 1)],
                        rhs=s_zrt[:, 512 * j : 512 * (j + 1)],
                        start=True,
                        stop=True,
                    )
                    tch = t[:, 512 * j : 512 * (j + 1)]
                    nc.scalar.activation(tch, ps[:], Act.Exp)
                    # kern * v_i * v_j in one fused DVE op
                    nc.vector.scalar_tensor_tensor(
                        out=tch,
                        in0=tch,
                        scalar=s_vcol[:, i : i + 1],
                        in1=s_vrow[:, 512 * j : 512 * (j + 1)],
                        op0=Alu.mult,
                        op1=Alu.mult,
                    )

                # f diagonal: += JITTER * eye (in place, before the f DMA)
                nc.vector.scalar_tensor_tensor(
                    out=t[:, c : c + 128],
                    in0=s_ey[:, 0:128],
                    scalar=float(JITTER),
                    in1=t[:, c : c + 128],
                    op0=Alu.mult,
                    op1=Alu.add,
                )
                nc.sync.dma_start(out=fj[128 * i : 128 * (i + 1), :], in_=t[:])

                # y diagonal chunk: yd = noise_i * eye + f_chunk (not in place,
                # so the y DMAs don't serialize behind the f DMA)
                yd = ydpool.tile([128, 512], f32)
                nc.vector.scalar_tensor_tensor(
                    out=yd[:],
                    in0=s_ey[:, 512 * m : 512 * (m + 1)],
                    scalar=s_ncol[:, i : i + 1],
                    in1=t[:, 512 * j0 : 512 * (j0 + 1)],
                    op0=Alu.mult,
                    op1=Alu.add,
                )
                rows = slice(128 * i, 128 * (i + 1))
                if j0 > 0:
                    nc.sync.dma_start(
                        out=yj[rows, 0 : 512 * j0], in_=t[:, 0 : 512 * j0]
                    )
                nc.sync.dma_start(
                    out=yj[rows, 512 * j0 : 512 * (j0 + 1)], in_=yd[:]
                )
                if j0 < NC4 - 1:
                    nc.sync.dma_start(
                        out=yj[rows, 512 * (j0 + 1) :], in_=t[:, 512 * (j0 + 1) :]
                    )

    nc.compile()
    return nc


def _get_compiled():
    global _COMPILED
    if _COMPILED is None:
        _COMPILED = _build_module()
    return _COMPILED


def _prep_inputs(tensor):
    """Host-side shard prep: per-batch augmented operands (all tiny, O(B*T))."""
    x = np.asarray(tensor, dtype=np.float32)
    assert x.shape == (B, T, 1 + D + 2), x.shape

    mean = np.ascontiguousarray(x[:, :, 0])
    z = x[:, :, 1 : 1 + D] / np.float32(np.sqrt(D))
    v = x[:, :, -2]
    noise = np.logaddexp(np.float32(0.0), x[:, :, -1]).astype(np.float32)
    sq = np.einsum("btd,btd->bt", z, z).astype(np.float32)

    ey = np.zeros((128, T), dtype=np.float32)
    r = np.arange(128)
    for k in range(NC4):
        ey[r, 512 * k + 128 * k + r] = 1.0

    in_maps = []
    for b in range(B):
        zlt = np.empty((K, T), dtype=np.float32)
        zrt = np.empty((K, T), dtype=np.float32)
        zlt[:D] = z[b].T
        zrt[:D] = z[b].T
        zlt[D] = -0.5 * sq[b]
        zrt[D] = 1.0
        zlt[D + 1] = 1.0
        zrt[D + 1] = -0.5 * sq[b]
        in_maps.append(
            {
                "zlt": zlt,
                "zrt": zrt,
                "vrow": np.ascontiguousarray(v[b][None, :]),
                "vcol": np.ascontiguousarray(v[b].reshape(NB, 128).T),
                "ncol": np.ascontiguousarray(noise[b].reshape(NB, 128).T),
                "ey": ey,
            }
        )
    return mean, in_maps


def kernel(tensor):
    from concourse.bass_utils import run_bass_kernel_spmd

    mean, in_maps = _prep_inputs(tensor)
    nc = _get_compiled()
    res = run_bass_kernel_spmd(nc, in_maps, list(range(B)))
    f_cov_j = np.stack([res.results[c]["fj"] for c in range(B)])
    y_cov_j = np.stack([res.results[c]["yj"] for c in range(B)])
    return mean, f_cov_j, y_cov_j


# revision 4
# speedup vs baseline: 1.1357x; 1.1357x over previous
"""Trainium2 Bass kernel for the KvvGaussianLayer problem.

Computes, from tensor (B=8, T=2048, F=67) with F = [mean, z(64), v, noise]:
    mean    = tensor[:, :, 0]                                  (B, T)
    f_cov_j = exp(-0.5*||z_i/8 - z_j/8||^2) * v_i * v_j + J*I  (B, T, T)
    y_cov_j = f_cov_j + softplus(noise) * I                    (B, T, T)

Sharding: data-parallel over batch B across the 8 NeuronCores (batch b on
core b); each core builds its own (T, T) covariances independently.

Device strategy (per core):
  - Host augments z with 2 extra contraction rows so a single K=66 matmul
    produces quad[i,j] = z_i.z_j - 0.5||z_i||^2 - 0.5||z_j||^2 directly.
  - PE: 64 matmuls (128x512 out tiles, K=66, fp32) -> PSUM
  - ACT: exp(PSUM) -> SBUF
  - DVE: one fused scalar_tensor_tensor: (kern * v_i) * v_j, plus diagonal
    jitter/noise adds using an eye tile.
  - DMA out f and y as 1MB row-block transfers (memory-bound problem: the
    32 MB of per-core output writes are the roofline).
"""

import numpy as np

B = 8
T = 2048
D = 64
K = D + 2          # augmented contraction dim
JITTER = 1e-06
NB = T // 128      # 16 row blocks per core
NC4 = T // 512     # 4 column chunks per row block

_COMPILED = None


def _build_module():
    """Build and compile the per-core Bass/Tile module (SPMD: same program
    on all 8 cores, different data)."""
    import concourse.bacc as bacc
    import concourse.tile as tile
    from concourse import mybir

    f32 = mybir.dt.float32
    Alu = mybir.AluOpType
    Act = mybir.ActivationFunctionType

    nc = bacc.Bacc("TRN2", target_bir_lowering=False, debug=False)

    f32r = mybir.dt.float32r
    zlt = nc.dram_tensor("zlt", [K, T], f32r, kind="ExternalInput").ap()
    zrt = nc.dram_tensor("zrt", [K, T], f32r, kind="ExternalInput").ap()
    vrow = nc.dram_tensor("vrow", [1, T], f32, kind="ExternalInput").ap()
    vcol = nc.dram_tensor("vcol", [128, NB], f32, kind="ExternalInput").ap()
    ncol = nc.dram_tensor("ncol", [128, NB], f32, kind="ExternalInput").ap()
    ey = nc.dram_tensor("ey", [128, T], f32, kind="ExternalInput").ap()
    fj = nc.dram_tensor("fj", [T, T], f32, kind="ExternalOutput").ap()
    yj = nc.dram_tensor("yj", [T, T], f32, kind="ExternalOutput").ap()

    with tile.TileContext(nc) as tc:
        with (
            tc.tile_pool(name="singles", bufs=1) as singles,
            tc.tile_pool(name="tpool", bufs=6) as tpool,
            tc.tile_pool(name="ydpool", bufs=6) as ydpool,
            tc.tile_pool(name="psum", bufs=8, space="PSUM") as psum,
        ):
            s_zlt = singles.tile([K, T], f32r)
            s_zrt = singles.tile([K, T], f32r)
            s_vrow = singles.tile([128, T], f32)
            s_vcol = singles.tile([128, NB], f32)
            s_ncol = singles.tile([128, NB], f32)
            s_ey = singles.tile([128, T], f32)

            nc.sync.dma_start(out=s_zlt[:], in_=zlt[:])
            nc.sync.dma_start(out=s_zrt[:], in_=zrt[:])
            nc.sync.dma_start(out=s_vrow[:], in_=vrow.to_broadcast([128, T]))
            nc.sync.dma_start(out=s_vcol[:], in_=vcol[:])
            nc.sync.dma_start(out=s_ncol[:], in_=ncol[:])
            nc.sync.dma_start(out=s_ey[:], in_=ey[:])

            for i in range(NB):
                t = tpool.tile([128, T], f32)
                j0 = i // 4          # column chunk containing the diagonal
                m = i % 4            # eye offset variant (diag at col 128*m)
                c = 128 * i          # diagonal column within the row block
                rows = slice(128 * i, 128 * (i + 1))

                for j in range(NC4):
                    ps = psum.tile([128, 512], f32)
                    nc.tensor.matmul(
                        ps[:],
                        lhsT=s_zlt[:, 128 * i : 128 * (i + 1)],
                        rhs=s_zrt[:, 512 * j : 512 * (j + 1)],
                        start=True,
                        stop=True,
                    )
                    cols = slice(512 * j, 512 * (j + 1))
                    tch = t[:, cols]
                    nc.scalar.activation(tch, ps[:], Act.Exp)
                    # kern * v_i * v_j in one fused DVE op
                    nc.vector.scalar_tensor_tensor(
                        out=tch,
                        in0=tch,
                        scalar=s_vcol[:, i : i + 1],
                        in1=s_vrow[:, cols],
                        op0=Alu.mult,
                        op1=Alu.mult,
                    )
                    if j == j0:
                        # f diagonal: += JITTER * eye (in place, pre f-DMA)
                        nc.vector.scalar_tensor_tensor(
                            out=t[:, c : c + 128],
                            in0=s_ey[:, 0:128],
                            scalar=float(JITTER),
                            in1=t[:, c : c + 128],
                            op0=Alu.mult,
                            op1=Alu.add,
                        )
                    # chunk-granularity DMAs: bytes start flowing as soon as
                    # the first chunk is ready (pipeline ramp, memory-bound)
                    nc.sync.dma_start(out=fj[rows, cols], in_=tch)
                    if j == j0:
                        # y diagonal chunk: yd = noise_i * eye + f_chunk (not
                        # in place, so y doesn't serialize behind the f DMA)
                        yd = ydpool.tile([128, 512], f32)
                        nc.vector.scalar_tensor_tensor(
                            out=yd[:],
                            in0=s_ey[:, 512 * m : 512 * (m + 1)],
                            scalar=s_ncol[:, i : i + 1],
                            in1=tch,
                            op0=Alu.mult,
                            op1=Alu.add,
                        )
                        nc.gpsimd.dma_start(out=yj[rows, cols], in_=yd[:])
                    else:
                        nc.gpsimd.dma_start(out=yj[rows, cols], in_=tch)

    nc.compile()
    return nc


def _get_compiled():
    global _COMPILED
    if _COMPILED is None:
        _COMPILED = _build_module()
    return _COMPILED


def _prep_inputs(tensor):
    """Host-side shard prep: per-batch augmented operands (all tiny, O(B*T))."""
    x = np.asarray(tensor, dtype=np.float32)
    assert x.shape == (B, T, 1 + D + 2), x.shape

    mean = np.ascontiguousarray(x[:, :, 0])
    z = x[:, :, 1 : 1 + D] / np.float32(np.sqrt(D))
    v = x[:, :, -2]
    noise = np.logaddexp(np.float32(0.0), x[:, :, -1]).astype(np.float32)
    sq = np.einsum("btd,btd->bt", z, z).astype(np.float32)

    ey = np.zeros((128, T), dtype=np.float32)
    r = np.arange(128)
    for k in range(NC4):
        ey[r, 512 * k + 128 * k + r] = 1.0

    in_maps = []
    for b in range(B):
        zlt = np.empty((K, T), dtype=np.float32)
        zrt = np.empty((K, T), dtype=np.float32)
        zlt[:D] = z[b].T
        zrt[:D] = z[b].T
        zlt[D] = -0.5 * sq[b]
        zrt[D] = 1.0
        zlt[D + 1] = 1.0
        zrt[D + 1] = -0.5 * sq[b]
        in_maps.append(
            {
                "zlt": zlt,
                "zrt": zrt,
                "vrow": np.ascontiguousarray(v[b][None, :]),
                "vcol": np.ascontiguousarray(v[b].reshape(NB, 128).T),
                "ncol": np.ascontiguousarray(noise[b].reshape(NB, 128).T),
                "ey": ey,
            }
        )
    return mean, in_maps


def kernel(tensor):
    from concourse.bass_utils import run_bass_kernel_spmd

    mean, in_maps = _prep_inputs(tensor)
    nc = _get_compiled()
    res = run_bass_kernel_spmd(nc, in_maps, list(range(B)))
    f_cov_j = np.stack([res.results[c]["fj"] for c in range(B)])
    y_cov_j = np.stack([res.results[c]["yj"] for c in range(B)])
    return mean, f_cov_j, y_cov_j


# revision 8
# speedup vs baseline: 1.1494x; 1.0120x over previous
"""Trainium2 Bass kernel for the KvvGaussianLayer problem.

Computes, from tensor (B=8, T=2048, F=67) with F = [mean, z(64), v, noise]:
    mean    = tensor[:, :, 0]                                  (B, T)
    f_cov_j = exp(-0.5*||z_i/8 - z_j/8||^2) * v_i * v_j + J*I  (B, T, T)
    y_cov_j = f_cov_j + softplus(noise) * I                    (B, T, T)

Sharding: data-parallel over batch B across the 8 NeuronCores (batch b on
core b); each core builds its own (T, T) covariances independently.

Device strategy (per core, memory-bound: 32 MB of output writes/core is the
roofline):
  - Host augments z with 2 extra contraction rows so a single K=66 matmul
    produces quad[i,j] = z_i.z_j - 0.5||z_i||^2 - 0.5||z_j||^2 directly.
  - PE: 64 matmuls (128x512 out tiles, K=66, fp32r = 1 cyc/row) -> PSUM
  - ACT: exp(PSUM) -> SBUF
  - DVE: one fused scalar_tensor_tensor: (kern * v_i) * v_j, plus diagonal
    jitter/noise adds against an on-device eye tile (affine_select).
  - f DMAs on SyncE (HWDGE), y DMAs on GpSimd (SWDGE) so neither sequencer
    saturates; chunk (256 KB) DMAs during ramp-up, 1 MB row DMAs after.
"""

import numpy as np

B = 8
T = 2048
D = 64
K = D + 2          # augmented contraction dim
JITTER = 1e-06
NB = T // 128      # 16 row blocks per core
NC4 = T // 512     # 4 column chunks per row block
RAMP = 16          # row blocks using chunk-granularity output DMAs

_COMPILED = None


def _build_module():
    """Build and compile the per-core Bass/Tile module (SPMD: same program
    on all 8 cores, different data)."""
    import concourse.bacc as bacc
    import concourse.tile as tile
    from concourse import mybir

    f32 = mybir.dt.float32
    f32r = mybir.dt.float32r
    Alu = mybir.AluOpType
    Act = mybir.ActivationFunctionType

    nc = bacc.Bacc("TRN2", target_bir_lowering=False, debug=False)

    zlt = nc.dram_tensor("zlt", [K, T], f32r, kind="ExternalInput").ap()
    zrt = nc.dram_tensor("zrt", [K, T], f32r, kind="ExternalInput").ap()
    vrow = nc.dram_tensor("vrow", [1, T], f32, kind="ExternalInput").ap()
    vcol = nc.dram_tensor("vcol", [128, NB], f32, kind="ExternalInput").ap()
    ncol = nc.dram_tensor("ncol", [128, NB], f32, kind="ExternalInput").ap()
    fj = nc.dram_tensor("fj", [T, T], f32, kind="ExternalOutput").ap()
    yj = nc.dram_tensor("yj", [T, T], f32, kind="ExternalOutput").ap()

    with tile.TileContext(nc) as tc:
        with (
            tc.tile_pool(name="singles", bufs=1) as singles,
            tc.tile_pool(name="tpool", bufs=6) as tpool,
            tc.tile_pool(name="ydpool", bufs=6) as ydpool,
            tc.tile_pool(name="psum", bufs=8, space="PSUM") as psum,
        ):
            s_zlt = singles.tile([K, T], f32r)
            s_zrt = singles.tile([K, T], f32r)
            s_vrow1 = singles.tile([1, T], f32)
            s_vrow = singles.tile([128, T], f32)
            s_vcol = singles.tile([128, NB], f32)
            s_ncol = singles.tile([128, NB], f32)
            s_one = singles.tile([128, 512], f32)
            s_ey = singles.tile([128, T], f32)

            nc.sync.dma_start(out=s_vrow1[:], in_=vrow[:])
            nc.sync.dma_start(out=s_vcol[:], in_=vcol[:])
            nc.sync.dma_start(out=s_zlt[:], in_=zlt[:])
            # chunk loads: the first matmul only waits for its own zrt slice
            for k in range(NC4):
                nc.sync.dma_start(
                    out=s_zrt[:, 512 * k : 512 * (k + 1)],
                    in_=zrt[:, 512 * k : 512 * (k + 1)],
                )
            nc.sync.dma_start(out=s_ncol[:], in_=ncol[:])

            # broadcast v along partitions on the (otherwise idle) GpSimd
            nc.gpsimd.partition_broadcast(s_vrow[:], s_vrow1[:])

            # eye strips on device: chunk k holds eye(128) at column 128*k
            nc.gpsimd.memset(s_one[:], 1.0)
            for k in range(NC4):
                nc.gpsimd.affine_select(
                    out=s_ey[:, 512 * k : 512 * (k + 1)],
                    in_=s_one[:],
                    pattern=[[1, 512]],
                    compare_op=Alu.is_equal,
                    fill=0.0,
                    base=-128 * k,
                    channel_multiplier=-1,
                )

            for i in range(NB):
                t = tpool.tile([128, T], f32)
                j0 = i // 4          # column chunk containing the diagonal
                m = i % 4            # eye offset variant (diag at col 128*m)
                c = 128 * i          # diagonal column within the row block
                rows = slice(128 * i, 128 * (i + 1))

                yd = None
                # diag chunk last: its extra eye/yd DVE work then never gates
                # the block's first output bytes
                for j in [x for x in range(NC4) if x != j0] + [j0]:
                    ps = psum.tile([128, 512], f32)
                    nc.tensor.matmul(
                        ps[:],
                        lhsT=s_zlt[:, 128 * i : 128 * (i + 1)],
                        rhs=s_zrt[:, 512 * j : 512 * (j + 1)],
                        start=True,
                        stop=True,
                    )
                    cols = slice(512 * j, 512 * (j + 1))
                    tch = t[:, cols]
                    nc.scalar.activation(tch, ps[:], Act.Exp)
                    # kern * v_i * v_j in one fused DVE op
                    nc.vector.scalar_tensor_tensor(
                        out=tch,
                        in0=tch,
                        scalar=s_vcol[:, i : i + 1],
                        in1=s_vrow[:, cols],
                        op0=Alu.mult,
                        op1=Alu.mult,
                    )
                    if j == j0:
                        # f diagonal: += JITTER * eye (in place, pre f-DMA)
                        nc.vector.scalar_tensor_tensor(
                            out=t[:, c : c + 128],
                            in0=s_ey[:, 0:128],
                            scalar=float(JITTER),
                            in1=t[:, c : c + 128],
                            op0=Alu.mult,
                            op1=Alu.add,
                        )
                        # y diagonal chunk: yd = noise_i * eye + f_chunk (not
                        # in place, so y doesn't serialize behind the f DMA)
                        yd = ydpool.tile([128, 512], f32)
                        nc.vector.scalar_tensor_tensor(
                            out=yd[:],
                            in0=s_ey[:, 512 * m : 512 * (m + 1)],
                            scalar=s_ncol[:, i : i + 1],
                            in1=tch,
                            op0=Alu.mult,
                            op1=Alu.add,
                        )
                    if i < RAMP:
                        # ramp-up: chunk DMAs so bytes flow immediately
                        nc.sync.dma_start(out=fj[rows, cols], in_=tch)
                        if j == j0:
                            nc.gpsimd.dma_start(out=yj[rows, cols], in_=yd[:])
                        else:
                            nc.gpsimd.dma_start(out=yj[rows, cols], in_=tch)

                if i >= RAMP:
                    # steady state: big DMAs (1 MB f; y split around the
                    # diagonal chunk) for minimum per-DMA overhead
                    nc.sync.dma_start(out=fj[rows, :], in_=t[:])
                    if j0 > 0:
                        nc.gpsimd.dma_start(
                            out=yj[rows, 0 : 512 * j0], in_=t[:, 0 : 512 * j0]
                        )
                    nc.gpsimd.dma_start(
                        out=yj[rows, 512 * j0 : 512 * (j0 + 1)], in_=yd[:]
                    )
                    if j0 < NC4 - 1:
                        nc.gpsimd.dma_start(
                            out=yj[rows, 512 * (j0 + 1) :],
                            in_=t[:, 512 * (j0 + 1) :],
                        )

    nc.compile()
    return nc


def _get_compiled():
    global _COMPILED
    if _COMPILED is None:
        _COMPILED = _build_module()
    return _COMPILED


def _prep_inputs(tensor):
    """Host-side shard prep: per-batch augmented operands (all tiny, O(B*T))."""
    x = np.asarray(tensor, dtype=np.float32)
    assert x.shape == (B, T, 1 + D + 2), x.shape

    mean = np.ascontiguousarray(x[:, :, 0])
    z = x[:, :, 1 : 1 + D] / np.float32(np.sqrt(D))
    v = x[:, :, -2]
    noise = np.logaddexp(np.float32(0.0), x[:, :, -1]).astype(np.float32)
    sq = np.einsum("btd,btd->bt", z, z).astype(np.float32)

    in_maps = []
    for b in range(B):
        zlt = np.empty((K, T), dtype=np.float32)
        zrt = np.empty((K, T), dtype=np.float32)
        zlt[:D] = z[b].T
        zrt[:D] = z[b].T
        zlt[D] = -0.5 * sq[b]
        zrt[D] = 1.0
        zlt[D + 1] = 1.0
        zrt[D + 1] = -0.5 * sq[b]
        in_maps.append(
            {
                "zlt": zlt,
                "zrt": zrt,
                "vrow": np.ascontiguousarray(v[b][None, :]),
                "vcol": np.ascontiguousarray(v[b].reshape(NB, 128).T),
                "ncol": np.ascontiguousarray(noise[b].reshape(NB, 128).T),
            }
        )
    return mean, in_maps


def kernel(tensor):
    from concourse.bass_utils import run_bass_kernel_spmd

    mean, in_maps = _prep_inputs(tensor)
    nc = _get_compiled()
    res = run_bass_kernel_spmd(nc, in_maps, list(range(B)))
    f_cov_j = np.stack([res.results[c]["fj"] for c in range(B)])
    y_cov_j = np.stack([res.results[c]["yj"] for c in range(B)])
    return mean, f_cov_j, y_cov_j
